# revision 11
# baseline (speedup 1.0000x reference)
# kernel.py — Bidirectional masked-GRU-with-predictor on 8 Trainium2 NeuronCores.
#
# Problem (reference.py): B=128, T=1024, H=512
#   per step, per direction:
#     x_in = where(mask, predictor(h), x)            predictor: Linear(H,H)->ReLU->Linear(H,1)->Tanh
#     h    = GRUCell(h, x_in)                        PyTorch gate order (r, z, n)
#   output [B, T, 2H] = concat(fwd hidden states, time-reversed bwd hidden states)
#
# Sharding: 8 cores = 8 batch groups of 16; EACH core runs BOTH directions
# (fwd + bwd) interleaved.  The kernel is weight-load bound on the PE
# (LDWEIGHTS streams ~26.7ns per 128-col fp16 block), so batch-splitting is
# free; interleaving the two independent direction recurrences hides each
# direction's long serial gate chain (relu->dot->tanh->gi->sigmoid->...->h')
# under the other direction's PE burst.
#
# On-core layout ("feature-major, chunk-in-free"):
#   h^T kept as [128 partitions = feature%128, (j,b)] where j = feature//128
#   (4 chunks), b = local batch (16).  Stationary = W^T 128x128 blocks
#   (values pre-scaled by 256 so an fp8 variant stays in the normal range;
#   activations divide by 256 via the ACT scale).  Biases for r/z/GIN ride in
#   the K=3 gi matmuls (rhs rows = [pred*m, x*(1-m), ones]); n/PH biases via
#   one E4 bias matmul each.  fp16 matmul inputs + fp32 PSUM accumulate.

import numpy as np

B, T, H = 128, 1024, 512
NCORES = 8
BL = B // NCORES     # 16: batch per core; both directions per core
KC = H // 128        # 4 contraction chunks
MC = (3 * H + H) // 128  # 16 output chunks (w_hh 12 + p_w1 4)
U_DEF = 32           # time steps per For_i iteration
WSCALE = 256.0       # stationary pre-scale (exact power of two)

_cache = {}


def _build_program(t_steps=T, u_steps=U_DEF, bl=BL, n_cores=NCORES):
    import concourse.bacc as bacc
    import concourse.bass as bass
    import concourse.tile as tile
    from concourse.tile import add_dep_helper
    from concourse import mybir

    f16 = mybir.dt.float16
    f32 = mybir.dt.float32
    w_dt = f16

    nc = bacc.Bacc(
        "TRN2",
        target_bir_lowering=False,
        debug=False,
        enable_asserts=False,
        num_devices=n_cores,
    )

    # ---- DRAM tensors (per-core data; same names on every core) ----
    d_wt = [nc.dram_tensor(f"wt{d}", [128, MC * KC * 128], w_dt,
                           kind="ExternalInput").ap() for d in range(2)]
    d_gi = [nc.dram_tensor(f"gil{d}", [2, 12 * 128], f16,
                           kind="ExternalInput").ap() for d in range(2)]
    d_bc = [nc.dram_tensor(f"bc{d}", [4, 5 * 128], f16,
                           kind="ExternalInput").ap() for d in range(2)]
    d_a = [nc.dram_tensor(f"a{d}", [t_steps, bl], f16,
                          kind="ExternalInput").ap() for d in range(2)]
    d_m = [nc.dram_tensor(f"m{d}", [t_steps, bl], f16,
                          kind="ExternalInput").ap() for d in range(2)]
    d_e4 = nc.dram_tensor("e4", [4, KC * bl], f16, kind="ExternalInput").ap()
    d_pw2 = nc.dram_tensor("pw2t", [128, KC], f16, kind="ExternalInput").ap()
    d_pb2 = nc.dram_tensor("pb2", [1, 1], f32, kind="ExternalInput").ap()
    d_out = [nc.dram_tensor(f"outl{d}", [t_steps, 128, KC, bl], f16,
                            kind="ExternalOutput").ap() for d in range(2)]

    Tanh = mybir.ActivationFunctionType.Tanh
    Sigmoid = mybir.ActivationFunctionType.Sigmoid
    SC = 1.0 / WSCALE

    with tile.TileContext(nc) as tc:
        import contextlib

        with contextlib.ExitStack() as ctx:
            consts = ctx.enter_context(tc.tile_pool(name="consts", bufs=1))
            psum = ctx.enter_context(tc.tile_pool(name="psum", bufs=1, space="PSUM"))
            work = ctx.enter_context(tc.tile_pool(name="work", bufs=2))
            io = ctx.enter_context(tc.tile_pool(name="io", bufs=2))

            # ---- constant preload ----
            WT, GIL, BC = [], [], []
            for d in range(2):
                WT.append(consts.tile([128, MC * KC * 128], w_dt, tag=f"WT{d}",
                                      name=f"WT{d}"))
                GIL.append(consts.tile([2, 12 * 128], f16, tag=f"GIL{d}",
                                       name=f"GIL{d}"))
                BC.append(consts.tile([4, 5 * 128], f16, tag=f"BC{d}",
                                      name=f"BC{d}"))
                nc.sync.dma_start(out=WT[d], in_=d_wt[d])
                nc.sync.dma_start(out=GIL[d], in_=d_gi[d])
                nc.sync.dma_start(out=BC[d], in_=d_bc[d])
            E4 = consts.tile([4, KC * bl], f16, tag="E4")
            PW2 = consts.tile([128, KC], f16, tag="PW2")
            PB2 = consts.tile([1, 1], f32, tag="PB2")
            for dst, src in ((E4, d_e4), (PW2, d_pw2), (PB2, d_pb2)):
                nc.sync.dma_start(out=dst, in_=src)

            # persistent ping-pong hidden state per direction, fp16, [128,(j,b)]
            h_tiles = []
            for d in range(2):
                hp = [consts.tile([128, KC * bl], f16, tag=f"h{d}{p}",
                                  name=f"h{d}{p}") for p in range(2)]
                nc.vector.memset(hp[0], 0.0)
                nc.vector.memset(hp[1], 0.0)
                h_tiles.append(hp)

            # PSUM accumulators: one bank per concurrently-live accumulation
            # region (start=True clears has_written bank-wide; PE-write +
            # DVE/ACT-read of one bank is a fatal collision).  Per direction:
            # G_r, G_z, G_n own a bank each; PH -> PRD -> GIN share the 4th
            # bank (their writes/reads are naturally serial within a step).
            # 2 directions x 4 banks = all 8 banks.  PREN lives in SBUF.
            W_ = KC * bl
            def mk_psum(tagd):
                g_r = psum.tile([128, W_], f32, tag=f"G_r{tagd}",
                                name=f"G_r{tagd}")
                g_z = psum.tile([128, W_], f32, tag=f"G_z{tagd}",
                                name=f"G_z{tagd}")
                g_n = psum.tile([128, W_], f32, tag=f"G_n{tagd}",
                                name=f"G_n{tagd}")
                phb = psum.tile([128, 2 * W_ + bl], f32, tag=f"PHB{tagd}",
                                name=f"PHB{tagd}")
                return {
                    "G_r": g_r, "G_z": g_z, "G_n": g_n,
                    "PH": phb[:, 0:W_],
                    "GIN": phb[:, W_:2 * W_],
                    "PRD": phb[0:1, 2 * W_:2 * W_ + bl],
                }
            P = [mk_psum(f"{d}") for d in range(2)]

            def w_block(d, m, k):
                bi = m * KC + k
                return WT[d][:, bi * 128:(bi + 1) * 128]

            state = {"prev": None}

            def pe_chain(first, last):
                # force PE issue order (ordering only, no extra sync)
                if state["prev"] is not None and first is not None:
                    add_dep_helper(first.ins, state["prev"].ins, sync=False)
                if last is not None:
                    state["prev"] = last

            def emit_w_region(d, base_m, region, h_cur, has_gi, bias_col):
                # start=True clears has_written for the WHOLE bank, so each
                # region is opened by exactly one E4 bias matmul spanning it;
                # every other matmul accumulates (start=False).
                first = nc.tensor.matmul(
                    region, BC[d][:, bias_col * 128:(bias_col + 1) * 128],
                    E4, start=True, stop=False, skip_group_check=True,
                )
                last = first
                for j in range(KC):
                    m = base_m + j
                    for k in range(KC):
                        last = nc.tensor.matmul(
                            region[:, j * bl:(j + 1) * bl],
                            w_block(d, m, k),
                            h_cur[:, k * bl:(k + 1) * bl],
                            start=False,
                            stop=(not has_gi and k == KC - 1),
                            skip_group_check=True,
                        )
                pe_chain(first, last)

            def emit_gi(d, g_idx, region, gi_rhs):
                # K=2 matmuls: region[:, j] += w_ih_g[j] (x) (tmp + a)
                first = last = None
                for j in range(KC):
                    gj = g_idx * KC + j
                    last = nc.tensor.matmul(
                        region[:, j * bl:(j + 1) * bl],
                        GIL[d][:, gj * 128:(gj + 1) * 128],
                        gi_rhs,
                        start=False, stop=True, skip_group_check=True,
                    )
                    if first is None:
                        first = last
                pe_chain(first, last)

            def step(u, d, S2, MB, t_dyn):
                h_cur = h_tiles[d][u % 2]
                h_new = h_tiles[d][(u + 1) % 2]
                R = P[d]
                gi_rhs = S2[:, u * bl:(u + 1) * bl]

                # --- PE stream (order: PH, W_r, PRD, W_z, W_n, gi_r, gi_z, GIN)
                emit_w_region(d, 12, R["PH"], h_cur, has_gi=False, bias_col=3)

                relu = work.tile([128, KC * bl], f16, tag=f"relu{d}")
                nc.vector.tensor_scalar_max(relu, R["PH"], 0.0)

                emit_w_region(d, 0, R["G_r"], h_cur, has_gi=True, bias_col=0)

                prd_f = prd_l = None
                for k in range(KC):
                    prd_l = nc.tensor.matmul(
                        R["PRD"], PW2[:, k:k + 1], relu[:, k * bl:(k + 1) * bl],
                        start=(k == 0), stop=(k == KC - 1),
                        skip_group_check=True,
                    )
                    if prd_f is None:
                        prd_f = prd_l
                pe_chain(prd_f, prd_l)

                pred = work.tile([1, bl], f16, tag=f"pred{d}")
                nc.scalar.activation(out=pred, in_=R["PRD"], func=Tanh,
                                     bias=PB2[:, :], scale=SC)
                nc.vector.tensor_mul(
                    S2[0:1, u * bl:(u + 1) * bl], pred,
                    MB[0:1, u * bl:(u + 1) * bl],
                )

                emit_w_region(d, 4, R["G_z"], h_cur, has_gi=True, bias_col=1)
                emit_w_region(d, 8, R["G_n"], h_cur, has_gi=False, bias_col=2)

                # GIN region: bias opener (start=True over the GIN slice of
                # bank D) then K=2 gi accumulation.
                gin_f = nc.tensor.matmul(
                    R["GIN"], BC[d][:, 4 * 128:5 * 128], E4,
                    start=True, stop=False, skip_group_check=True)
                pe_chain(gin_f, gin_f)
                emit_gi(d, 0, R["G_r"], gi_rhs)
                emit_gi(d, 1, R["G_z"], gi_rhs)
                emit_gi(d, 2, R["GIN"], gi_rhs)

                # --- gate math ---
                r_sb = work.tile([128, KC * bl], f16, tag=f"r_sb{d}")
                nc.scalar.activation(out=r_sb, in_=R["G_r"], func=Sigmoid,
                                     scale=SC)
                z_sb = work.tile([128, KC * bl], f16, tag=f"z_sb{d}")
                nc.scalar.activation(out=z_sb, in_=R["G_z"], func=Sigmoid,
                                     scale=SC)

                u_n = work.tile([128, KC * bl], f16, tag=f"u_n{d}")
                nc.vector.tensor_mul(u_n, r_sb, R["G_n"])
                pren = work.tile([128, KC * bl], f32, tag=f"pren{d}")
                nc.vector.tensor_add(pren, u_n, R["GIN"])
                n_sb = work.tile([128, KC * bl], f16, tag=f"n_sb{d}")
                nc.scalar.activation(out=n_sb, in_=pren, func=Tanh,
                                     scale=SC)

                # h' = z*h - (z-1)*n ;  t1 = z*h can start right after z_sb
                t1 = work.tile([128, KC * bl], f16, tag=f"t1{d}")
                nc.vector.tensor_mul(t1, z_sb, h_cur)
                t2 = work.tile([128, KC * bl], f16, tag=f"t2{d}")
                nc.vector.scalar_tensor_tensor(
                    out=t2, in0=z_sb, scalar=1.0, in1=n_sb,
                    op0=mybir.AluOpType.subtract, op1=mybir.AluOpType.mult,
                )
                nc.vector.tensor_sub(h_new, t1, t2)

                # stream h' out:  outl[t, p, j, b]
                dst = d_out[d][bass.ds(t_dyn, 1)].rearrange(
                    "o p j b -> (o p) j b")
                nc.sync.dma_start(
                    out=dst, in_=h_new.rearrange("p (j b) -> p j b", b=bl)
                )

            n_blocks = t_steps // u_steps
            with tc.For_i(
                0, n_blocks, 1, hint_engines=(mybir.EngineType.PE,)
            ) as iv:
                S2s, MBs = [], []
                for d in range(2):
                    S2 = io.tile([2, u_steps * bl], f16, tag=f"S2{d}",
                                 name=f"S2{d}")
                    MB = io.tile([1, u_steps * bl], f16, tag=f"MB{d}",
                                 name=f"MB{d}")
                    nc.sync.dma_start(
                        out=S2[1:2, :].rearrange("p (u b) -> p u b", b=bl),
                        in_=d_a[d][bass.ds(iv * u_steps, u_steps)].unsqueeze(0),
                    )
                    nc.sync.dma_start(
                        out=MB[0:1, :].rearrange("p (u b) -> p u b", b=bl),
                        in_=d_m[d][bass.ds(iv * u_steps, u_steps)].unsqueeze(0),
                    )
                    S2s.append(S2)
                    MBs.append(MB)
                for u in range(u_steps):
                    for d in range(2):
                        step(u, d, S2s[d], MBs[d], iv * u_steps + u)

    nc.compile()
    return nc


def _prep_core_inputs(inputs, core, t_steps=T, bl=BL):
    """Build the per-core input map (numpy) for core id `core`."""
    f16 = np.float16
    sl = slice(core * bl, (core + 1) * bl)

    x = np.asarray(inputs["x"], np.float32)[:, :, 0]      # [B, T]
    msk = np.asarray(inputs["mask"]).astype(np.float32)[:, :, 0]
    p_w1 = np.asarray(inputs["p_w1"], np.float32)
    p_b1 = np.asarray(inputs["p_b1"], np.float32)
    p_w2 = np.asarray(inputs["p_w2"], np.float32)
    p_b2 = np.asarray(inputs["p_b2"], np.float32)

    out = {}
    for d, pfx in enumerate(("wf", "wb")):
        w_ih = np.asarray(inputs[f"{pfx}_ih"], np.float32)[:, 0]   # [3H]
        w_hh = np.asarray(inputs[f"{pfx}_hh"], np.float32)         # [3H, H]
        b_ih = np.asarray(inputs[f"b{pfx[1]}_ih"], np.float32)
        b_hh = np.asarray(inputs[f"b{pfx[1]}_hh"], np.float32)

        xs = x[sl].T.copy()      # [T, bl]
        ms = msk[sl].T.copy()
        if d == 1:
            xs = xs[::-1].copy()
            ms = ms[::-1].copy()
        out[f"a{d}"] = (xs * (1.0 - ms)).astype(f16)[:t_steps]
        out[f"m{d}"] = ms.astype(f16)[:t_steps]

        W = np.concatenate([w_hh, p_w1], axis=0) * WSCALE    # [2048, 512]
        Wr = W.reshape(MC, 128, KC, 128)                     # [m, c, k, p]
        out[f"wt{d}"] = (
            Wr.transpose(3, 0, 2, 1).reshape(128, MC * KC * 128).astype(f16)
        )

        # gi stationaries: per (gate g, chunk j) a [2,128] block, both
        # rows = w_ih chunk; contract with [tmp; a] rows of S2.
        gil = np.stack([w_ih, w_ih], axis=0) * WSCALE            # [2, 1536]
        out[f"gil{d}"] = gil.astype(f16)

        bias_regions = [
            b_ih[0:H] + b_hh[0:H],          # r
            b_ih[H:2 * H] + b_hh[H:2 * H],  # z
            b_hh[2 * H:3 * H],              # n: b_hh only
            p_b1,                           # PH
            b_ih[2 * H:3 * H],              # GIN: b_ih_n
        ]
        out[f"bc{d}"] = (np.concatenate(
            [br.reshape(KC, 128) for br in bias_regions], axis=1
        ) * WSCALE).astype(f16)             # [4, 5*128]

    e4 = np.zeros((KC, KC, bl), np.float32)
    for j in range(KC):
        e4[j, j, :] = 1.0
    out["e4"] = e4.reshape(KC, KC * bl).astype(f16)
    out["pw2t"] = p_w2[0].reshape(KC, 128).T.astype(f16).copy()
    out["pb2"] = p_b2.reshape(1, 1).astype(np.float32)
    return out


def _assemble(results, t_steps=T, bl=BL):
    """results: list of 8 per-core dicts with 'outl{0,1}' [T,128,KC,bl] fp16."""
    out = np.zeros((B, t_steps, 2 * H), np.float32)
    for core in range(NCORES):
        sl = slice(core * bl, (core + 1) * bl)
        for d in range(2):
            arr = np.asarray(results[core][f"outl{d}"], np.float16)
            arr = arr.astype(np.float32)
            # [t, p, j, b] -> [b, t, j, p] -> [b, t, 512]
            arr = arr.transpose(3, 0, 2, 1).reshape(bl, t_steps, H)
            if d == 1:
                arr = arr[:, ::-1]
            out[sl, :, d * H:(d + 1) * H] = arr
    return out


def kernel(**inputs):
    from concourse.bass_utils import run_bass_kernel_spmd

    key = (T, U_DEF, BL)
    if key not in _cache:
        _cache[key] = _build_program(T, U_DEF, BL)
    nc = _cache[key]

    in_maps = [_prep_core_inputs(inputs, c) for c in range(NCORES)]
    res = run_bass_kernel_spmd(
        nc, in_maps, core_ids=list(range(NCORES)), trace=False
    )
    return _assemble(res.results)


# revision 13
# speedup vs baseline: 1.1247x; 1.1247x over previous
# kernel.py — Bidirectional masked-GRU-with-predictor on 8 Trainium2 NeuronCores.
#
# Problem (reference.py): B=128, T=1024, H=512
#   per step, per direction:
#     x_in = where(mask, predictor(h), x)            predictor: Linear(H,H)->ReLU->Linear(H,1)->Tanh
#     h    = GRUCell(h, x_in)                        PyTorch gate order (r, z, n)
#   output [B, T, 2H] = concat(fwd hidden states, time-reversed bwd hidden states)
#
# Sharding: 8 cores = 8 batch groups of 16; EACH core runs BOTH directions
# (fwd + bwd) interleaved.  The kernel is weight-load bound on the PE
# (LDWEIGHTS streams ~26.7ns per 128-col fp16 block), so batch-splitting is
# free; interleaving the two independent direction recurrences hides each
# direction's long serial gate chain (relu->dot->tanh->gi->sigmoid->...->h')
# under the other direction's PE burst.
#
# On-core layout ("feature-major, chunk-in-free"):
#   h^T kept as [128 partitions = feature%128, (j,b)] where j = feature//128
#   (4 chunks), b = local batch (16).  Stationary = W^T 128x128 blocks
#   (values pre-scaled by 256 so an fp8 variant stays in the normal range;
#   activations divide by 256 via the ACT scale).  Biases for r/z/GIN ride in
#   the K=3 gi matmuls (rhs rows = [pred*m, x*(1-m), ones]); n/PH biases via
#   one E4 bias matmul each.  fp16 matmul inputs + fp32 PSUM accumulate.

import numpy as np

B, T, H = 128, 1024, 512
NCORES = 8
BL = B // NCORES     # 16: batch per core; both directions per core
KC = H // 128        # 4 contraction chunks
MC = (3 * H + H) // 128  # 16 output chunks (w_hh 12 + p_w1 4)
U_DEF = 32           # time steps per For_i iteration
WSCALE = 256.0       # stationary pre-scale (exact power of two)

_cache = {}


def _build_program(t_steps=T, u_steps=U_DEF, bl=BL, n_cores=NCORES):
    import concourse.bacc as bacc
    import concourse.bass as bass
    import concourse.tile as tile
    from concourse.tile import add_dep_helper
    from concourse import mybir

    f16 = mybir.dt.float16
    f32 = mybir.dt.float32
    w_dt = f16

    nc = bacc.Bacc(
        "TRN2",
        target_bir_lowering=False,
        debug=False,
        enable_asserts=False,
        num_devices=n_cores,
    )

    # ---- DRAM tensors (per-core data; same names on every core) ----
    d_wt = [nc.dram_tensor(f"wt{d}", [128, MC * KC * 128], w_dt,
                           kind="ExternalInput").ap() for d in range(2)]
    d_gi = [nc.dram_tensor(f"gil{d}", [128, 12 * 128], f16,
                           kind="ExternalInput").ap() for d in range(2)]
    d_bc = [nc.dram_tensor(f"bc{d}", [128, 5 * 128], f16,
                           kind="ExternalInput").ap() for d in range(2)]
    d_a = [nc.dram_tensor(f"a{d}", [t_steps, bl], f16,
                          kind="ExternalInput").ap() for d in range(2)]
    d_m = [nc.dram_tensor(f"m{d}", [t_steps, bl], f16,
                          kind="ExternalInput").ap() for d in range(2)]
    d_e4 = nc.dram_tensor("e4", [128, KC * bl], f16, kind="ExternalInput").ap()
    d_pw2 = nc.dram_tensor("pw2t", [128, KC * 128], f16,
                           kind="ExternalInput").ap()
    d_pb2 = nc.dram_tensor("pb2", [1, 1], f32, kind="ExternalInput").ap()
    d_out = [nc.dram_tensor(f"outl{d}", [t_steps, 128, KC, bl], f16,
                            kind="ExternalOutput").ap() for d in range(2)]

    Tanh = mybir.ActivationFunctionType.Tanh
    Sigmoid = mybir.ActivationFunctionType.Sigmoid
    SC = 1.0 / WSCALE

    with tile.TileContext(nc) as tc:
        import contextlib

        with contextlib.ExitStack() as ctx:
            consts = ctx.enter_context(tc.tile_pool(name="consts", bufs=1))
            psum = ctx.enter_context(tc.tile_pool(name="psum", bufs=1, space="PSUM"))
            work = ctx.enter_context(tc.tile_pool(name="work", bufs=2))
            io = ctx.enter_context(tc.tile_pool(name="io", bufs=2))

            # ---- constant preload ----
            WT, GIL, BC = [], [], []
            for d in range(2):
                WT.append(consts.tile([128, MC * KC * 128], w_dt, tag=f"WT{d}",
                                      name=f"WT{d}"))
                GIL.append(consts.tile([128, 12 * 128], f16, tag=f"GIL{d}",
                                       name=f"GIL{d}"))
                BC.append(consts.tile([128, 5 * 128], f16, tag=f"BC{d}",
                                      name=f"BC{d}"))
                nc.sync.dma_start(out=WT[d], in_=d_wt[d])
                nc.sync.dma_start(out=GIL[d], in_=d_gi[d])
                nc.sync.dma_start(out=BC[d], in_=d_bc[d])
            E4 = consts.tile([128, KC * bl], f16, tag="E4")
            PW2 = consts.tile([128, KC * 128], f16, tag="PW2")
            PB2 = consts.tile([1, 1], f32, tag="PB2")
            for dst, src in ((E4, d_e4), (PW2, d_pw2), (PB2, d_pb2)):
                nc.sync.dma_start(out=dst, in_=src)

            # persistent ping-pong hidden state per direction, fp16, [128,(j,b)]
            h_tiles = []
            for d in range(2):
                hp = [consts.tile([128, KC * bl], f16, tag=f"h{d}{p}",
                                  name=f"h{d}{p}") for p in range(2)]
                nc.vector.memset(hp[0], 0.0)
                nc.vector.memset(hp[1], 0.0)
                h_tiles.append(hp)

            # PSUM accumulators: one bank per concurrently-live accumulation
            # region (start=True clears has_written bank-wide; PE-write +
            # DVE/ACT-read of one bank is a fatal collision).  Per direction:
            # G_r, G_z, G_n own a bank each; PH -> PRD -> GIN share the 4th
            # bank (their writes/reads are naturally serial within a step).
            # 2 directions x 4 banks = all 8 banks.  PREN lives in SBUF.
            W_ = KC * bl
            def mk_psum(tagd):
                g_r = psum.tile([128, W_], f32, tag=f"G_r{tagd}",
                                name=f"G_r{tagd}")
                g_z = psum.tile([128, W_], f32, tag=f"G_z{tagd}",
                                name=f"G_z{tagd}")
                g_n = psum.tile([128, W_], f32, tag=f"G_n{tagd}",
                                name=f"G_n{tagd}")
                phb = psum.tile([128, 2 * W_ + bl], f32, tag=f"PHB{tagd}",
                                name=f"PHB{tagd}")
                return {
                    "G_r": g_r, "G_z": g_z, "G_n": g_n,
                    "PH": phb[:, 0:W_],
                    "GIN": phb[:, W_:2 * W_],
                    "PRD": phb[:, 2 * W_:2 * W_ + bl],
                }
            P = [mk_psum(f"{d}") for d in range(2)]

            def w_block(d, m, k):
                bi = m * KC + k
                return WT[d][:, bi * 128:(bi + 1) * 128]

            state = {"prev": None}

            def pe_chain(first, last):
                # force PE issue order (ordering only, no extra sync)
                if state["prev"] is not None and first is not None:
                    add_dep_helper(first.ins, state["prev"].ins, sync=False)
                if last is not None:
                    state["prev"] = last

            def emit_w_region(d, base_m, region, h_cur, has_gi, bias_col):
                # start=True clears has_written for the WHOLE bank, so each
                # region is opened by exactly one E4 bias matmul spanning it;
                # every other matmul accumulates (start=False).
                first = nc.tensor.matmul(
                    region, BC[d][:, bias_col * 128:(bias_col + 1) * 128],
                    E4, start=True, stop=False, skip_group_check=True,
                )
                last = first
                for j in range(KC):
                    m = base_m + j
                    for k in range(KC):
                        last = nc.tensor.matmul(
                            region[:, j * bl:(j + 1) * bl],
                            w_block(d, m, k),
                            h_cur[:, k * bl:(k + 1) * bl],
                            start=False,
                            stop=(not has_gi and k == KC - 1),
                            skip_group_check=True,
                        )
                pe_chain(first, last)

            def emit_gi(d, g_idx, region, gi_rhs):
                # K=2 matmuls: region[:, j] += w_ih_g[j] (x) (tmp + a)
                first = last = None
                for j in range(KC):
                    gj = g_idx * KC + j
                    last = nc.tensor.matmul(
                        region[:, j * bl:(j + 1) * bl],
                        GIL[d][:, gj * 128:(gj + 1) * 128],
                        gi_rhs,
                        start=False, stop=True, skip_group_check=True,
                    )
                    if first is None:
                        first = last
                pe_chain(first, last)

            def step(u, d, S2, MB, t_dyn):
                h_cur = h_tiles[d][u % 2]
                h_new = h_tiles[d][(u + 1) % 2]
                R = P[d]
                gi_rhs = S2[:, u * bl:(u + 1) * bl]

                # --- PE stream (order: PH, W_r, PRD, W_z, W_n, gi_r, gi_z, GIN)
                emit_w_region(d, 12, R["PH"], h_cur, has_gi=False, bias_col=3)

                relu = work.tile([128, KC * bl], f16, tag=f"relu{d}")
                nc.vector.tensor_scalar_max(relu, R["PH"], 0.0)

                emit_w_region(d, 0, R["G_r"], h_cur, has_gi=True, bias_col=0)

                prd_f = prd_l = None
                for k in range(KC):
                    prd_l = nc.tensor.matmul(
                        R["PRD"], PW2[:, k * 128:(k + 1) * 128],
                        relu[:, k * bl:(k + 1) * bl],
                        start=(k == 0), stop=(k == KC - 1),
                        skip_group_check=True,
                    )
                    if prd_f is None:
                        prd_f = prd_l
                pe_chain(prd_f, prd_l)

                pred = work.tile([1, bl], f16, tag=f"pred{d}")
                nc.scalar.activation(out=pred, in_=R["PRD"][0:1, :], func=Tanh,
                                     bias=PB2[:, :], scale=SC)
                nc.vector.tensor_mul(
                    S2[0:1, u * bl:(u + 1) * bl], pred,
                    MB[0:1, u * bl:(u + 1) * bl],
                )

                emit_w_region(d, 4, R["G_z"], h_cur, has_gi=True, bias_col=1)
                emit_w_region(d, 8, R["G_n"], h_cur, has_gi=False, bias_col=2)

                # GIN region: bias opener (start=True over the GIN slice of
                # bank D) then K=2 gi accumulation.
                gin_f = nc.tensor.matmul(
                    R["GIN"], BC[d][:, 4 * 128:5 * 128], E4,
                    start=True, stop=False, skip_group_check=True)
                pe_chain(gin_f, gin_f)
                emit_gi(d, 0, R["G_r"], gi_rhs)
                emit_gi(d, 1, R["G_z"], gi_rhs)
                emit_gi(d, 2, R["GIN"], gi_rhs)

                # --- gate math ---
                r_sb = work.tile([128, KC * bl], f16, tag=f"r_sb{d}")
                nc.scalar.activation(out=r_sb, in_=R["G_r"], func=Sigmoid,
                                     scale=SC)
                z_sb = work.tile([128, KC * bl], f16, tag=f"z_sb{d}")
                nc.scalar.activation(out=z_sb, in_=R["G_z"], func=Sigmoid,
                                     scale=SC)

                u_n = work.tile([128, KC * bl], f16, tag=f"u_n{d}")
                nc.vector.tensor_mul(u_n, r_sb, R["G_n"])
                pren = work.tile([128, KC * bl], f32, tag=f"pren{d}")
                nc.vector.tensor_add(pren, u_n, R["GIN"])
                n_sb = work.tile([128, KC * bl], f16, tag=f"n_sb{d}")
                nc.scalar.activation(out=n_sb, in_=pren, func=Tanh,
                                     scale=SC)

                # h' = z*h - (z-1)*n ;  t1 = z*h can start right after z_sb
                t1 = work.tile([128, KC * bl], f16, tag=f"t1{d}")
                nc.vector.tensor_mul(t1, z_sb, h_cur)
                t2 = work.tile([128, KC * bl], f16, tag=f"t2{d}")
                nc.vector.scalar_tensor_tensor(
                    out=t2, in0=z_sb, scalar=1.0, in1=n_sb,
                    op0=mybir.AluOpType.subtract, op1=mybir.AluOpType.mult,
                )
                nc.vector.tensor_sub(h_new, t1, t2)

                # stream h' out:  outl[t, p, j, b]
                dst = d_out[d][bass.ds(t_dyn, 1)].rearrange(
                    "o p j b -> (o p) j b")
                nc.sync.dma_start(
                    out=dst, in_=h_new.rearrange("p (j b) -> p j b", b=bl)
                )

            n_blocks = t_steps // u_steps
            with tc.For_i(
                0, n_blocks, 1, hint_engines=(mybir.EngineType.PE,)
            ) as iv:
                S2s, MBs = [], []
                for d in range(2):
                    S2 = io.tile([128, u_steps * bl], f16, tag=f"S2{d}",
                                 name=f"S2{d}")
                    nc.vector.memset(S2, 0.0)
                    MB = io.tile([1, u_steps * bl], f16, tag=f"MB{d}",
                                 name=f"MB{d}")
                    nc.sync.dma_start(
                        out=S2[1:2, :].rearrange("p (u b) -> p u b", b=bl),
                        in_=d_a[d][bass.ds(iv * u_steps, u_steps)].unsqueeze(0),
                    )
                    nc.sync.dma_start(
                        out=MB[0:1, :].rearrange("p (u b) -> p u b", b=bl),
                        in_=d_m[d][bass.ds(iv * u_steps, u_steps)].unsqueeze(0),
                    )
                    S2s.append(S2)
                    MBs.append(MB)
                for u in range(u_steps):
                    for d in range(2):
                        step(u, d, S2s[d], MBs[d], iv * u_steps + u)

    nc.compile()
    return nc


def _prep_core_inputs(inputs, core, t_steps=T, bl=BL):
    """Build the per-core input map (numpy) for core id `core`."""
    f16 = np.float16
    sl = slice(core * bl, (core + 1) * bl)

    x = np.asarray(inputs["x"], np.float32)[:, :, 0]      # [B, T]
    msk = np.asarray(inputs["mask"]).astype(np.float32)[:, :, 0]
    p_w1 = np.asarray(inputs["p_w1"], np.float32)
    p_b1 = np.asarray(inputs["p_b1"], np.float32)
    p_w2 = np.asarray(inputs["p_w2"], np.float32)
    p_b2 = np.asarray(inputs["p_b2"], np.float32)

    out = {}
    for d, pfx in enumerate(("wf", "wb")):
        w_ih = np.asarray(inputs[f"{pfx}_ih"], np.float32)[:, 0]   # [3H]
        w_hh = np.asarray(inputs[f"{pfx}_hh"], np.float32)         # [3H, H]
        b_ih = np.asarray(inputs[f"b{pfx[1]}_ih"], np.float32)
        b_hh = np.asarray(inputs[f"b{pfx[1]}_hh"], np.float32)

        xs = x[sl].T.copy()      # [T, bl]
        ms = msk[sl].T.copy()
        if d == 1:
            xs = xs[::-1].copy()
            ms = ms[::-1].copy()
        out[f"a{d}"] = (xs * (1.0 - ms)).astype(f16)[:t_steps]
        out[f"m{d}"] = ms.astype(f16)[:t_steps]

        W = np.concatenate([w_hh, p_w1], axis=0) * WSCALE    # [2048, 512]
        Wr = W.reshape(MC, 128, KC, 128)                     # [m, c, k, p]
        out[f"wt{d}"] = (
            Wr.transpose(3, 0, 2, 1).reshape(128, MC * KC * 128).astype(f16)
        )

        # gi stationaries: per (gate g, chunk j) a [128,128] block with
        # rows 0,1 = w_ih chunk (rest zero); contract with S2 (rows 2+ zero).
        gil = np.zeros((128, 12 * 128), np.float32)
        gil[0] = gil[1] = w_ih * WSCALE
        out[f"gil{d}"] = gil.astype(f16)

        bias_regions = [
            b_ih[0:H] + b_hh[0:H],          # r
            b_ih[H:2 * H] + b_hh[H:2 * H],  # z
            b_hh[2 * H:3 * H],              # n: b_hh only
            p_b1,                           # PH
            b_ih[2 * H:3 * H],              # GIN: b_ih_n
        ]
        bc = np.zeros((128, 5 * 128), np.float32)
        bc[:KC] = np.concatenate(
            [br.reshape(KC, 128) for br in bias_regions], axis=1) * WSCALE
        out[f"bc{d}"] = bc.astype(f16)      # [128, 5*128], rows 4+ zero

    e4 = np.zeros((128, KC * bl), np.float32)
    for j in range(KC):
        e4[j, j * bl:(j + 1) * bl] = 1.0
    out["e4"] = e4.astype(f16)
    pw2 = np.zeros((128, KC * 128), np.float32)
    for k in range(KC):
        pw2[:, k * 128] = p_w2[0][k * 128:(k + 1) * 128]
    out["pw2t"] = pw2.astype(f16)
    out["pb2"] = p_b2.reshape(1, 1).astype(np.float32)
    return out


def _assemble(results, t_steps=T, bl=BL):
    """results: list of 8 per-core dicts with 'outl{0,1}' [T,128,KC,bl] fp16."""
    out = np.zeros((B, t_steps, 2 * H), np.float32)
    for core in range(NCORES):
        sl = slice(core * bl, (core + 1) * bl)
        for d in range(2):
            arr = np.asarray(results[core][f"outl{d}"], np.float16)
            arr = arr.astype(np.float32)
            # [t, p, j, b] -> [b, t, j, p] -> [b, t, 512]
            arr = arr.transpose(3, 0, 2, 1).reshape(bl, t_steps, H)
            if d == 1:
                arr = arr[:, ::-1]
            out[sl, :, d * H:(d + 1) * H] = arr
    return out


def kernel(**inputs):
    from concourse.bass_utils import run_bass_kernel_spmd

    key = (T, U_DEF, BL)
    if key not in _cache:
        _cache[key] = _build_program(T, U_DEF, BL)
    nc = _cache[key]

    in_maps = [_prep_core_inputs(inputs, c) for c in range(NCORES)]
    res = run_bass_kernel_spmd(
        nc, in_maps, core_ids=list(range(NCORES)), trace=False
    )
    return _assemble(res.results)
